# revision 36
# baseline (speedup 1.0000x reference)
"""Trainium2 Bass kernel for the mca_g2l sparse-attention module — single core.

The multi-core dispatch floor through the axon relay (~1.1-1.6 ms/exec for
2-8 cores vs ~0.3 ms for 1 core) dwarfs this problem's compute (~22.6 G MACs
~= 0.6 ms of f16 PE time), so the whole module runs on ONE NeuronCore with
the 8 heads looped — no collectives, no cross-core staging.

Layout: feature-major ("^T": [feature, tokens]) everywhere; attention is
key-major (S^T [keys, queries]) so softmax denominators come from
ones-matmuls and the AV / ave-branch matmuls need no attention transpose.

Weights are baked into the NEFF as inline constants (loaded to HBM once at
model load). Per-exec inputs: x^T (f16, 8.4 MB) + cls_score (8 KB).

Precision: f16 projections / raw sims / output linears; f32r QK^T attention;
bf16 softmax-exp tiles (range: exp(25) overflows f16); f32 PSUM accumulation
throughout. End-to-end rel err ~5e-4 vs the f32 reference (gate 2e-2).

Per-head SBUF residents: full x^T (64 KB/partition, f16), per-head P tiles
(bf16), v tiles; normalized v (raw-sim input) and token-major raw v
(ave-branch support) stream through DRAM for the after-the-head-loop phases.
"""

import hashlib

import numpy as np

import concourse.bacc as bacc
import concourse.mybir as mybir
import concourse.tile as tile
from concourse.masks import make_identity

F32 = mybir.dt.float32
F32R = mybir.dt.float32r
F16 = mybir.dt.float16
BF16 = mybir.dt.bfloat16
AF = mybir.ActivationFunctionType

N_CORES = 1
H = 8
N1 = 512
N2 = 2048
C = 1024
HD = 128
SCALE = 25.0
KT = N2 // 128          # 16 key tiles of 128
TT = N2 // 512          # 4 token tiles of 512
CC = C // 128           # 8 contraction chunks per projection
CL = 2 * C // 128       # 16 contraction chunks for the output linears

B = ("cls", "reg")
W_SLOT = {("q", "cls"): 0, ("k", "cls"): 1, ("v", "cls"): 2,
          ("q", "reg"): 3, ("k", "reg"): 4, ("v", "reg"): 5}

# output rows (feature-major): [ave_cls | out_cls | ave_reg | out_reg]
O_AVE = {"cls": 0, "reg": 3072}
O_LIN = {"cls": 1024, "reg": 4096}


def _prefer_combined_act_table():
    """Make natural_log_exp_and_others the only table serving Exp/Ln so the
    ATL-insertion pass settles on that ONE table for this kernel's whole
    Exp/Ln/Square/Copy mix (no per-instruction table reloads). Table order —
    and hence the act_func_set ids walrus resolves against act_info.json —
    is preserved; only the chooser's view of which table serves Exp/Ln is
    narrowed."""
    import concourse.hw_specs as hs
    if getattr(bacc, "_act_tables_patched", False):
        return
    orig = hs.get_activation_tables

    def narrowed(arch):
        t = orig(arch)
        key = "natural_log_exp_and_others"
        if key not in t:
            return t
        strip = {AF.Exp, AF.Ln}
        return {k: (v if k == key else set(v) - strip) for k, v in t.items()}

    bacc.get_activation_tables = narrowed
    bacc._act_tables_patched = True


def build_nc(consts: dict):
    _prefer_combined_act_table()
    nc = bacc.Bacc("TRN2", target_bir_lowering=False, debug=False,
                   num_devices=N_CORES)

    # ---- kernel I/O ----
    xin = nc.dram_tensor("xin", [2 * C, N2], F16, kind="ExternalInput")
    scr = nc.dram_tensor("scr", [8, 256], F32, kind="ExternalInput")
    out_t = nc.dram_tensor("out", [6144, 512], F32, kind="ExternalOutput")
    oap = out_t.ap()

    # ---- baked-in weights ----
    wqkv_t = nc.inline_tensor(consts["wqkv"], name="wqkv")  # [48*128, 1024] f16
    wlin_t = nc.inline_tensor(consts["wlin"], name="wlin")  # [32*128, 2048] f16
    bias_t = nc.inline_tensor(consts["bias"], name="bias")  # [128, 32] f32
    wqkv_ap, wlin_ap = wqkv_t.ap(), bias_t and wlin_t.ap()

    with tile.TileContext(nc) as tc:
        with tc.tile_pool(name="dram", bufs=1, space="DRAM") as dramp, \
             tc.tile_pool(name="const", bufs=1) as constp, \
             tc.tile_pool(name="persist", bufs=1) as persist:

            # DRAM staging for the post-head-loop phases
            vn_d = dramp.tile([2, 128, H, N2], F16, name="vn_d")   # normalized v^T
            vt_d = dramp.tile([2, H, 128, KT, 128], BF16, name="vt_d")  # token-major v

            # ---- constants ----
            ones_f = constp.tile([128, 1], F32, name="ones_f")
            nc.vector.memset(ones_f[:], 1.0)
            ones = constp.tile([128, 1], F32R, name="ones")
            nc.vector.tensor_copy(ones[:], ones_f[:])
            ones16 = constp.tile([128, 1], BF16, name="ones16")
            nc.vector.tensor_copy(ones16[:], ones_f[:])
            ident_f = constp.tile([128, 128], F32, name="ident_f")
            make_identity(nc, ident_f[:])
            ident = constp.tile([128, 128], F32R, name="ident")
            nc.vector.tensor_copy(ident[:], ident_f[:])
            score_s = constp.tile([1, N2], F32, name="score_s")
            nc.sync.dma_start(score_s[:].rearrange("o (f n) -> o f n", f=8),
                              scr.ap())
            bias_s = constp.tile([128, 32], F32, name="bias_s")
            nc.sync.dma_start(bias_s[:], bias_t.ap())

            # ---- persistent accumulators across the head loop ----
            XG = {b: persist.tile([128, H, N1], F16, name=f"XG_{b}",
                                  tag=f"XG_{b}") for b in B}     # AV outputs
            VG = {b: persist.tile([128, H, N1], F16, name=f"VG_{b}",
                                  tag=f"VG_{b}") for b in B}     # v^T[:, :N1]
            avs = persist.tile([128, KT, N1], BF16, name="avs")  # sum_h attn_avg
            nc.vector.memset(avs[:], 0.0)

            # =========== head loop: projections + attention ===========
            with tc.tile_pool(name="hpool", bufs=1) as hpool, \
                 tc.tile_pool(name="wpool", bufs=2) as wpool, \
                 tc.tile_pool(name="xs", bufs=3) as xs, \
                 tc.tile_pool(name="htmp", bufs=2) as htmp, \
                 tc.tile_pool(name="hrow", bufs=1) as hrow, \
                 tc.tile_pool(name="psA", bufs=1, space="PSUM") as psA, \
                 tc.tile_pool(name="psN", bufs=1, space="PSUM") as psN, \
                 tc.tile_pool(name="psT", bufs=1, space="PSUM") as psT, \
                 tc.tile_pool(name="psS", bufs=2, space="PSUM") as psS, \
                 tc.tile_pool(name="psX", bufs=1, space="PSUM") as psX:

                for h in range(H):
                    # all 6 projection weight slices of this head in one DMA
                    w6 = wpool.tile([128, 6, CC, HD], F16, name="w6", tag="w6")
                    nc.sync.dma_start(
                        w6[:],
                        wqkv_ap[h * 768:(h + 1) * 768, :]
                        .rearrange("(s p) m -> p s m", p=128))
                    w_s = {(t, b): w6[:, s] for (t, b), s in W_SLOT.items()}

                    kS = {b: hpool.tile([128, KT, 128], F16, name=f"kS_{b}",
                                        tag=f"kS_{b}") for b in B}
                    qN = {b: hpool.tile([128, N1], F16, name=f"qN_{b}",
                                        tag=f"qN_{b}") for b in B}
                    vTok = {b: hpool.tile([128, KT, 128], BF16, name=f"vTok_{b}",
                                          tag=f"vTok_{b}") for b in B}
                    P = {b: hpool.tile([128, KT, N1], BF16, name=f"P_{b}",
                                       tag=f"P_{b}") for b in B}

                    def inv_norm(ps):
                        # 1/||col|| = exp(-0.5*ln(sum col^2)); Square/Ln/Exp
                        # all live in the natural_log_exp activation table
                        # (preferred above) -> no table reloads.
                        sq = htmp.tile([128, 512], F32R, name="sq", tag="sq")
                        nc.scalar.activation(sq[:], ps[:], AF.Square)
                        nsq = psN.tile([1, 512], F32, name="nsq", tag="nsq")
                        nc.tensor.matmul(nsq[:], ones[:], sq[:],
                                         start=True, stop=True)
                        st = htmp.tile([1, 512], F32, name="st", tag="st")
                        nc.scalar.activation(st[:], nsq[:], AF.Ln)
                        rt = htmp.tile([1, 512], F32, name="rt", tag="rt")
                        nc.scalar.activation(rt[:], st[:], AF.Exp, scale=-0.5)
                        return rt

                    def bcast(row):
                        bt = htmp.tile([128, 512], F32, name="bc", tag="bc")
                        nc.gpsimd.partition_broadcast(bt[:], row[:])
                        return bt

                    # ---------------- projections ----------------
                    for ib, b in enumerate(B):
                        for tt in range(TT):
                            # all 8 C-chunks of this token tile in one DMA
                            xtile = xs.tile([128, CC, 512], F16, name="xt",
                                            tag="xt")
                            nc.sync.dma_start(
                                xtile[:],
                                xin.ap()[ib * C:(ib + 1) * C,
                                         tt * 512:(tt + 1) * 512]
                                .rearrange("(c p) n -> p c n", p=128))

                            pk = psA.tile([128, 512], F32, name="pk", tag="pk")
                            pv = psA.tile([128, 512], F32, name="pv", tag="pv")
                            pq = (psS.tile([128, 512], F32, name="s", tag="s")
                                  if tt == 0 else None)
                            for c in range(CC):
                                nc.tensor.matmul(pk[:], w_s["k", b][:, c, :],
                                                 xtile[:, c, :], start=(c == 0),
                                                 stop=(c == CC - 1))
                                nc.tensor.matmul(pv[:], w_s["v", b][:, c, :],
                                                 xtile[:, c, :], start=(c == 0),
                                                 stop=(c == CC - 1))
                                if tt == 0:
                                    nc.tensor.matmul(
                                        pq[:], w_s["q", b][:, c, :],
                                        xtile[:, c, :], start=(c == 0),
                                        stop=(c == CC - 1))

                            if tt == 0:
                                rq = inv_norm(pq)
                                nc.vector.tensor_mul(qN[b][:], pq[:],
                                                     bcast(rq)[:])

                            tsl = slice(tt * 4, (tt + 1) * 4)

                            # k: fold SCALE (and cls_score) and 1/|k| in
                            rk = inv_norm(pk)
                            fk = htmp.tile([1, 512], F32, name="fk", tag="fk")
                            nc.vector.tensor_scalar_mul(fk[:], rk[:], SCALE)
                            if b == "cls":
                                nc.vector.tensor_mul(
                                    fk[:], fk[:],
                                    score_s[:, tt * 512:(tt + 1) * 512])
                            nc.vector.tensor_mul(kS[b][:, tsl, :], pk[:],
                                                 bcast(fk)[:])

                            # v: normalized + raw + token-major transposes
                            rv = inv_norm(pv)
                            vN_sb = htmp.tile([128, 512], F16, name="vN",
                                              tag="vN")
                            nc.vector.tensor_mul(vN_sb[:], pv[:], bcast(rv)[:])
                            nc.sync.dma_start(
                                vn_d[ib][:, h, tt * 512:(tt + 1) * 512],
                                vN_sb[:])
                            vraw = htmp.tile([128, 512], F32R, name="vraw",
                                             tag="vraw")
                            nc.scalar.activation(vraw[:], pv[:], AF.Copy)
                            if tt == 0:
                                nc.vector.tensor_copy(VG[b][:, h, :], vraw[:])
                            tp4 = psT.tile([128, 4, 128], F32R, name="tp",
                                           tag="tp")
                            for j in range(4):
                                nc.tensor.transpose(
                                    tp4[:, j, :], vraw[:, j * 128:(j + 1) * 128],
                                    ident[:])
                            nc.vector.tensor_copy(vTok[b][:, tsl, :], tp4[:])

                        nc.sync.dma_start(vt_d[ib, h], vTok[b][:])

                    # ---------------- attention ----------------
                    Rhalf = {}
                    for b in B:
                        dacc = psN.tile([1, N1], F32, name="dacc", tag="nsq")
                        for kt in range(KT):
                            s = psS.tile([128, N1], F32, name="s", tag="s")
                            nc.tensor.matmul(s[:], kS[b][:, kt, :], qN[b][:],
                                             start=True, stop=True)
                            p_t = P[b][:, kt, :]
                            nc.scalar.activation(p_t, s[:], AF.Exp)
                            nc.tensor.matmul(dacc[:], ones16[:], p_t,
                                             start=(kt == 0),
                                             stop=(kt == KT - 1))
                        d2 = htmp.tile([1, N1], F32, name="d2", tag="d2")
                        nc.vector.tensor_scalar_mul(d2[:], dacc[:], 2.0)
                        rh = htmp.tile([1, N1], F32, name="rh", tag="rh")
                        nc.vector.reciprocal(rh[:], d2[:])
                        Rhalf[b] = hrow.tile([128, N1], F32, name=f"Rh_{b}",
                                             tag=f"Rh_{b}")
                        nc.gpsimd.partition_broadcast(Rhalf[b][:], rh[:])

                    xacc = {b: psX.tile([128, N1], F32, name=f"x_{b}",
                                        tag=f"x_{b}") for b in B}
                    for kt in range(KT):
                        for b in B:
                            nc.vector.tensor_mul(P[b][:, kt, :], P[b][:, kt, :],
                                                 Rhalf[b][:])
                        # head-sum of the blended attention for the ave
                        # branch: accumulate on the (mostly idle) DMA engines
                        # so the Pool/DVE chains stay off the critical path
                        for b in B:
                            nc.gpsimd.dma_start(avs[:, kt, :], P[b][:, kt, :],
                                                accum_op=mybir.AluOpType.add)
                        for b in B:
                            for i2, b2 in enumerate(B):
                                nc.tensor.matmul(
                                    xacc[b][:], vTok[b][:, kt, :],
                                    P[b2][:, kt, :],
                                    start=(kt == 0 and i2 == 0),
                                    stop=(kt == KT - 1 and i2 == 1))
                    for b in B:
                        nc.vector.tensor_copy(XG[b][:, h, :], xacc[b][:])

            # =========== raw value-similarity masks ===========
            with tc.tile_pool(name="cpool", bufs=1) as cpool:
                msk = {b: cpool.tile([128, KT, N1], F16, name=f"msk_{b}",
                                     tag=f"msk_{b}") for b in B}
                with tc.tile_pool(name="ckey", bufs=3) as ckey, \
                     tc.tile_pool(name="rawps", bufs=2, space="PSUM") as rawps:
                    for ib, (b, thr) in enumerate((("cls", 0.75),
                                                   ("reg", 0.99))):
                        vnq = cpool.tile([128, H, N1], F16, name=f"vnq_{b}",
                                         tag=f"vnq_{b}")
                        nc.sync.dma_start(vnq[:], vn_d[ib][:, :, 0:N1])
                        for kt in range(KT):
                            vnk = ckey.tile([128, H, 128], F16, name="vnk",
                                            tag="vnk")
                            nc.sync.dma_start(
                                vnk[:],
                                vn_d[ib][:, :, kt * 128:(kt + 1) * 128])
                            rp = rawps.tile([128, N1], F32, name="raw",
                                            tag="raw")
                            for hh in range(H):
                                nc.tensor.matmul(rp[:], vnk[:, hh, :],
                                                 vnq[:, hh, :],
                                                 start=(hh == 0),
                                                 stop=(hh == H - 1))
                            nc.vector.tensor_scalar(
                                msk[b][:, kt, :], rp[:], 1.0 / H, thr,
                                mybir.AluOpType.mult, mybir.AluOpType.is_gt)

                # ---- masked exp of the head-averaged attention ----
                mes = cpool.tile([128, KT, N1], BF16, name="mes")
                meo = cpool.tile([128, KT, N1], BF16, name="meo")
                Rd = {}
                with tc.tile_pool(name="dps", bufs=1, space="PSUM") as dps:
                    dp = {b: dps.tile([1, N1], F32, name=f"dp_{b}",
                                      tag=f"dp_{b}") for b in B}
                    for kt in range(KT):
                        e_t = cpool.tile([128, N1], F32R, name="e_t",
                                         tag="e_t")
                        nc.scalar.activation(e_t[:], avs[:, kt, :], AF.Exp,
                                             scale=1.0 / H)
                        nc.vector.tensor_mul(mes[:, kt, :], e_t[:],
                                             msk["cls"][:, kt, :])
                        nc.vector.tensor_mul(meo[:, kt, :], mes[:, kt, :],
                                             msk["reg"][:, kt, :])
                        nc.tensor.matmul(dp["cls"][:], ones16[:],
                                         mes[:, kt, :],
                                         start=(kt == 0), stop=(kt == KT - 1))
                        nc.tensor.matmul(dp["reg"][:], ones16[:],
                                         meo[:, kt, :],
                                         start=(kt == 0), stop=(kt == KT - 1))
                    for b in B:
                        rr = cpool.tile([1, N1], F32, name=f"rr_{b}",
                                        tag=f"rr_{b}")
                        nc.vector.reciprocal(rr[:], dp[b][:])
                        Rd[b] = cpool.tile([128, N1], F32, name=f"Rd_{b}",
                                           tag=f"Rd_{b}")
                        nc.gpsimd.partition_broadcast(Rd[b][:], rr[:])

                # =========== output linears ===========
                with tc.tile_pool(name="lw", bufs=3) as lw, \
                     tc.tile_pool(name="linps", bufs=3, space="PSUM") as linps, \
                     tc.tile_pool(name="ltmp", bufs=3) as ltmp:
                    for ib, b in enumerate(B):
                        for m in range(16):
                            wl = lw.tile([128, CL, 128], F16, name="wl",
                                         tag="wl")
                            base = (ib * 16 + m) * 128
                            nc.sync.dma_start(wl[:],
                                              wlin_ap[base:base + 128, :])
                            op_ = linps.tile([128, N1], F32, name="olin",
                                             tag="olin")
                            for c in range(CL):
                                rhs = (XG[b][:, c, :] if c < CC
                                       else VG[b][:, c - CC, :])
                                nc.tensor.matmul(op_[:], wl[:, c, :], rhs,
                                                 start=(c == 0),
                                                 stop=(c == CL - 1))
                            osb = ltmp.tile([128, N1], F32, name="osb",
                                            tag="osb")
                            nc.vector.tensor_scalar_add(
                                osb[:], op_[:],
                                bias_s[:, ib * 16 + m:ib * 16 + m + 1])
                            r0 = O_LIN[b] + m * 128
                            nc.sync.dma_start(oap[r0:r0 + 128, :], osb[:])

                    # =========== ave branch ===========
                    MS = {"cls": mes, "reg": meo}
                    for ib, b in enumerate(B):
                        for h in range(H):
                            vt = lw.tile([128, KT, 128], BF16, name="vt",
                                         tag="vt")
                            nc.sync.dma_start(vt[:], vt_d[ib, h][:])
                            ap_ = linps.tile([128, N1], F32, name="avep",
                                             tag="avep")
                            for kt in range(KT):
                                nc.tensor.matmul(ap_[:], vt[:, kt, :],
                                                 MS[b][:, kt, :],
                                                 start=(kt == 0),
                                                 stop=(kt == KT - 1))
                            asb = ltmp.tile([128, N1], F32, name="asb",
                                            tag="asb")
                            nc.vector.tensor_mul(asb[:], ap_[:], Rd[b][:])
                            r0 = O_AVE[b] + h * 128
                            nc.sync.dma_start(oap[r0:r0 + 128, :], asb[:])

    nc.finalize()
    return nc


def make_consts(inputs: dict) -> dict:
    """Host-side: pre-lay all weights into const arrays baked into the NEFF."""
    W_q = {"cls": np.asarray(inputs["W_q_cls"], np.float32),
           "reg": np.asarray(inputs["W_q_reg"], np.float32)}
    W_kv = {"cls": np.asarray(inputs["W_kv_cls"], np.float32),
            "reg": np.asarray(inputs["W_kv_reg"], np.float32)}
    W_l = {"cls": np.asarray(inputs["W_lin"], np.float32),
           "reg": np.asarray(inputs["W_lin_reg"], np.float32)}
    b_l = {"cls": np.asarray(inputs["b_lin"], np.float32),
           "reg": np.asarray(inputs["b_lin_reg"], np.float32)}

    wqkv = np.zeros((H * 6 * 128, CC * 128), np.float16)
    for h in range(H):
        hs = slice(h * HD, (h + 1) * HD)
        vs = slice(C + h * HD, C + (h + 1) * HD)
        for (t, b), s in W_SLOT.items():
            src = (W_q[b][:, hs] if t == "q" else
                   W_kv[b][:, hs] if t == "k" else W_kv[b][:, vs])   # [C, 128]
            lay = src.reshape(CC, 128, 128).transpose(1, 0, 2)       # [p, c, m]
            wqkv[(h * 6 + s) * 128:(h * 6 + s + 1) * 128] = \
                lay.reshape(128, CC * 128).astype(np.float16)

    wlin = np.zeros((2 * 16 * 128, CL * 128), np.float16)
    for ib, b in enumerate(B):
        for m in range(16):
            src = W_l[b][:, m * 128:(m + 1) * 128]                   # [2C, 128]
            lay = src.reshape(CL, 128, 128).transpose(1, 0, 2)       # [p, c, u]
            wlin[(ib * 16 + m) * 128:(ib * 16 + m + 1) * 128] = \
                lay.reshape(128, CL * 128).astype(np.float16)

    bias = np.zeros((128, 32), np.float32)
    for ib, b in enumerate(B):
        bias[:, ib * 16:(ib + 1) * 16] = b_l[b].reshape(16, 128).T

    return {"wqkv": wqkv, "wlin": wlin, "bias": bias}


def make_in_maps(inputs: dict) -> list[dict]:
    """Host-side staging: full x^T (f16) + cls_score, single core."""
    x_cls = np.asarray(inputs["x_cls"], np.float32)[0]      # [N2, C]
    x_reg = np.asarray(inputs["x_reg"], np.float32)[0]
    scr = np.asarray(inputs["cls_score"], np.float32).reshape(8, 256)
    xin = np.concatenate([np.ascontiguousarray(x_cls.T),
                          np.ascontiguousarray(x_reg.T)], 0).astype(np.float16)
    return [{"xin": xin, "scr": scr}]


def assemble(results: list[dict]) -> tuple[np.ndarray, np.ndarray]:
    out = results[0]["out"]
    cls_feature = np.ascontiguousarray(out[0:3072].T, dtype=np.float32)
    reg_feature = np.ascontiguousarray(out[3072:6144].T, dtype=np.float32)
    return cls_feature, reg_feature


_CACHE = {}


def _weights_digest(inputs: dict) -> str:
    hsh = hashlib.sha1()
    for k in ("W_q_cls", "W_kv_cls", "W_q_reg", "W_kv_reg",
              "W_lin", "b_lin", "W_lin_reg", "b_lin_reg"):
        hsh.update(np.ascontiguousarray(np.asarray(inputs[k], np.float32)).tobytes())
    return hsh.hexdigest()


def get_nc(inputs: dict | None = None):
    if inputs is not None:
        dig = _weights_digest(inputs)
        if _CACHE.get("digest") != dig:
            _CACHE.clear()
            _CACHE["digest"] = dig
            _CACHE["nc"] = build_nc(make_consts(inputs))
    return _CACHE["nc"]


class _Runner:
    """Cached jitted SPMD executor (mirrors bass2jax.run_bass_via_pjrt)."""

    def __init__(self, nc):
        import jax
        from jax.sharding import Mesh, PartitionSpec
        from jax.experimental.shard_map import shard_map
        from concourse.bass2jax import (_bass_exec_p, install_neuronx_cc_hook,
                                        partition_id_tensor)
        install_neuronx_cc_hook()
        self.jax = jax
        pname = nc.partition_id_tensor.name if nc.partition_id_tensor else None
        in_names, out_names, out_avals, zero_outs = [], [], [], []
        for alloc in nc.m.functions[0].allocations:
            if not isinstance(alloc, mybir.MemoryLocationSet):
                continue
            name = alloc.memorylocations[0].name
            if alloc.kind == "ExternalInput":
                if name != pname:
                    in_names.append(name)
            elif alloc.kind == "ExternalOutput":
                out_names.append(name)
                shape = tuple(alloc.tensor_shape)
                dtype = mybir.dt.np(alloc.dtype)
                out_avals.append(jax.core.ShapedArray(shape, dtype))
                zero_outs.append(np.zeros(shape, dtype))
        self.in_names, self.out_names = in_names, out_names
        self.out_avals, self.zero_outs = out_avals, zero_outs
        n_params, n_outs = len(in_names), len(out_names)
        all_in = in_names + out_names + ([pname] if pname else [])

        def _body(*args):
            operands = list(args)
            if pname is not None:
                operands.append(partition_id_tensor())
            return tuple(_bass_exec_p.bind(
                *operands, out_avals=tuple(out_avals), in_names=tuple(all_in),
                out_names=tuple(out_names), lowering_input_output_aliases=(),
                sim_require_finite=True, sim_require_nnan=True, nc=nc))

        devices = jax.devices()[:N_CORES]
        mesh = Mesh(np.asarray(devices), ("core",))
        self.fn = jax.jit(
            shard_map(_body, mesh=mesh,
                      in_specs=(PartitionSpec("core"),) * (n_params + n_outs),
                      out_specs=(PartitionSpec("core"),) * n_outs,
                      check_rep=False),
            keep_unused=True)

    def __call__(self, in_maps):
        n = N_CORES
        concat_in = [np.concatenate([np.asarray(in_maps[c][k]) for c in range(n)], 0)
                     for k in self.in_names]
        concat_zeros = [np.zeros((n * z.shape[0], *z.shape[1:]), z.dtype)
                        for z in self.zero_outs]
        outs = self.fn(*concat_in, *concat_zeros)
        self.jax.block_until_ready(outs)
        return [{name: np.asarray(outs[i]).reshape(n, *self.out_avals[i].shape)[c]
                 for i, name in enumerate(self.out_names)}
                for c in range(n)]


def get_runner():
    if "runner" not in _CACHE:
        _CACHE["runner"] = _Runner(get_nc())
    return _CACHE["runner"]


def kernel(**inputs) -> tuple[np.ndarray, np.ndarray]:
    get_nc(inputs)
    results = get_runner()(make_in_maps(inputs))
    return assemble(results)


# revision 37
# speedup vs baseline: 1.4118x; 1.4118x over previous
"""Trainium2 Bass kernel for the mca_g2l sparse-attention module — single core.

The multi-core dispatch floor through the axon relay (~1.1-1.6 ms/exec for
2-8 cores vs ~0.3 ms for 1 core) dwarfs this problem's compute (~22.6 G MACs
~= 0.6 ms of f16 PE time), so the whole module runs on ONE NeuronCore with
the 8 heads looped — no collectives, no cross-core staging.

Layout: feature-major ("^T": [feature, tokens]) everywhere; attention is
key-major (S^T [keys, queries]) so softmax denominators come from
ones-matmuls and the AV / ave-branch matmuls need no attention transpose.

Weights are baked into the NEFF as inline constants (loaded to HBM once at
model load). Per-exec inputs: x^T (f16, 8.4 MB) + cls_score (8 KB).

Precision: f16 projections / raw sims / output linears; f32r QK^T attention;
bf16 softmax-exp tiles (range: exp(25) overflows f16); f32 PSUM accumulation
throughout. End-to-end rel err ~5e-4 vs the f32 reference (gate 2e-2).

Per-head SBUF residents: full x^T (64 KB/partition, f16), per-head P tiles
(bf16), v tiles; normalized v (raw-sim input) and token-major raw v
(ave-branch support) stream through DRAM for the after-the-head-loop phases.
"""

import hashlib

import numpy as np

import concourse.bacc as bacc
import concourse.mybir as mybir
import concourse.tile as tile
from concourse.masks import make_identity

F32 = mybir.dt.float32
F32R = mybir.dt.float32r
F16 = mybir.dt.float16
BF16 = mybir.dt.bfloat16
AF = mybir.ActivationFunctionType

N_CORES = 1
H = 8
N1 = 512
N2 = 2048
C = 1024
HD = 128
SCALE = 25.0
KT = N2 // 128          # 16 key tiles of 128
TT = N2 // 512          # 4 token tiles of 512
CC = C // 128           # 8 contraction chunks per projection
CL = 2 * C // 128       # 16 contraction chunks for the output linears

B = ("cls", "reg")
W_SLOT = {("q", "cls"): 0, ("k", "cls"): 1, ("v", "cls"): 2,
          ("q", "reg"): 3, ("k", "reg"): 4, ("v", "reg"): 5}

# output rows (feature-major): [ave_cls | out_cls | ave_reg | out_reg]
O_AVE = {"cls": 0, "reg": 3072}
O_LIN = {"cls": 1024, "reg": 4096}


def _prefer_combined_act_table():
    """Make natural_log_exp_and_others the only table serving Exp/Ln so the
    ATL-insertion pass settles on that ONE table for this kernel's whole
    Exp/Ln/Square/Copy mix (no per-instruction table reloads). Table order —
    and hence the act_func_set ids walrus resolves against act_info.json —
    is preserved; only the chooser's view of which table serves Exp/Ln is
    narrowed."""
    import concourse.hw_specs as hs
    if getattr(bacc, "_act_tables_patched", False):
        return
    orig = hs.get_activation_tables

    def narrowed(arch):
        t = orig(arch)
        key = "natural_log_exp_and_others"
        if key not in t:
            return t
        strip = {AF.Exp, AF.Ln}
        return {k: (v if k == key else set(v) - strip) for k, v in t.items()}

    bacc.get_activation_tables = narrowed
    bacc._act_tables_patched = True


def build_nc(consts: dict):
    _prefer_combined_act_table()
    nc = bacc.Bacc("TRN2", target_bir_lowering=False, debug=False,
                   num_devices=N_CORES)

    # ---- kernel I/O ----
    xin = nc.dram_tensor("xin", [2 * C, N2], F16, kind="ExternalInput")
    scr = nc.dram_tensor("scr", [8, 256], F32, kind="ExternalInput")
    out_t = nc.dram_tensor("out", [6144, 512], F32, kind="ExternalOutput")
    oap = out_t.ap()

    # ---- baked-in weights ----
    wqkv_t = nc.inline_tensor(consts["wqkv"], name="wqkv")  # [48*128, 1024] f16
    wlin_t = nc.inline_tensor(consts["wlin"], name="wlin")  # [32*128, 2048] f16
    bias_t = nc.inline_tensor(consts["bias"], name="bias")  # [128, 32] f32
    wqkv_ap, wlin_ap = wqkv_t.ap(), bias_t and wlin_t.ap()

    with tile.TileContext(nc) as tc:
        with tc.tile_pool(name="dram", bufs=1, space="DRAM") as dramp, \
             tc.tile_pool(name="const", bufs=1) as constp, \
             tc.tile_pool(name="persist", bufs=1) as persist:

            # DRAM staging for the post-head-loop phases
            vn_d = dramp.tile([2, 128, H, N2], F16, name="vn_d")   # normalized v^T
            vt_d = dramp.tile([2, H, 128, KT, 128], BF16, name="vt_d")  # token-major v

            # ---- constants ----
            ones_f = constp.tile([128, 1], F32, name="ones_f")
            nc.vector.memset(ones_f[:], 1.0)
            ones = constp.tile([128, 1], F32R, name="ones")
            nc.vector.tensor_copy(ones[:], ones_f[:])
            ones16 = constp.tile([128, 1], BF16, name="ones16")
            nc.vector.tensor_copy(ones16[:], ones_f[:])
            ident_f = constp.tile([128, 128], F32, name="ident_f")
            make_identity(nc, ident_f[:])
            ident = constp.tile([128, 128], F32R, name="ident")
            nc.vector.tensor_copy(ident[:], ident_f[:])
            score_s = constp.tile([1, N2], F32, name="score_s")
            nc.sync.dma_start(score_s[:].rearrange("o (f n) -> o f n", f=8),
                              scr.ap())
            bias_s = constp.tile([128, 32], F32, name="bias_s")
            nc.sync.dma_start(bias_s[:], bias_t.ap())

            # ---- persistent accumulators across the head loop ----
            XG = {b: persist.tile([128, H, N1], F16, name=f"XG_{b}",
                                  tag=f"XG_{b}") for b in B}     # AV outputs
            VG = {b: persist.tile([128, H, N1], F16, name=f"VG_{b}",
                                  tag=f"VG_{b}") for b in B}     # v^T[:, :N1]
            avs = persist.tile([128, KT, N1], BF16, name="avs")  # sum_h attn_avg
            nc.vector.memset(avs[:], 0.0)

            # =========== head loop: projections + attention ===========
            with tc.tile_pool(name="hpool", bufs=1) as hpool, \
                 tc.tile_pool(name="wpool", bufs=2) as wpool, \
                 tc.tile_pool(name="xs", bufs=3) as xs, \
                 tc.tile_pool(name="htmp", bufs=2) as htmp, \
                 tc.tile_pool(name="hrow", bufs=1) as hrow, \
                 tc.tile_pool(name="psA", bufs=1, space="PSUM") as psA, \
                 tc.tile_pool(name="psN", bufs=1, space="PSUM") as psN, \
                 tc.tile_pool(name="psT", bufs=1, space="PSUM") as psT, \
                 tc.tile_pool(name="psS", bufs=2, space="PSUM") as psS, \
                 tc.tile_pool(name="psX", bufs=1, space="PSUM") as psX:

                for h in range(H):
                    # all 6 projection weight slices of this head in one DMA
                    w6 = wpool.tile([128, 6, CC, HD], F16, name="w6", tag="w6")
                    nc.sync.dma_start(
                        w6[:],
                        wqkv_ap[h * 768:(h + 1) * 768, :]
                        .rearrange("(s p) m -> p s m", p=128))
                    w_s = {(t, b): w6[:, s] for (t, b), s in W_SLOT.items()}

                    kS = {b: hpool.tile([128, KT, 128], F32R, name=f"kS_{b}",
                                        tag=f"kS_{b}") for b in B}
                    qN = {b: hpool.tile([128, N1], F32R, name=f"qN_{b}",
                                        tag=f"qN_{b}") for b in B}
                    vTok = {b: hpool.tile([128, KT, 128], BF16, name=f"vTok_{b}",
                                          tag=f"vTok_{b}") for b in B}
                    P = {b: hpool.tile([128, KT, N1], BF16, name=f"P_{b}",
                                       tag=f"P_{b}") for b in B}

                    def inv_norm(ps):
                        # 1/||col|| = exp(-0.5*ln(sum col^2)); Square/Ln/Exp
                        # all live in the natural_log_exp activation table
                        # (preferred above) -> no table reloads.
                        sq = htmp.tile([128, 512], F32R, name="sq", tag="sq")
                        nc.scalar.activation(sq[:], ps[:], AF.Square)
                        nsq = psN.tile([1, 512], F32, name="nsq", tag="nsq")
                        nc.tensor.matmul(nsq[:], ones[:], sq[:],
                                         start=True, stop=True)
                        st = htmp.tile([1, 512], F32, name="st", tag="st")
                        nc.scalar.activation(st[:], nsq[:], AF.Ln)
                        rt = htmp.tile([1, 512], F32, name="rt", tag="rt")
                        nc.scalar.activation(rt[:], st[:], AF.Exp, scale=-0.5)
                        return rt

                    def bcast(row):
                        bt = htmp.tile([128, 512], F32, name="bc", tag="bc")
                        nc.gpsimd.partition_broadcast(bt[:], row[:])
                        return bt

                    # ---------------- projections ----------------
                    for ib, b in enumerate(B):
                        for tt in range(TT):
                            # all 8 C-chunks of this token tile in one DMA
                            xtile = xs.tile([128, CC, 512], F16, name="xt",
                                            tag="xt")
                            nc.sync.dma_start(
                                xtile[:],
                                xin.ap()[ib * C:(ib + 1) * C,
                                         tt * 512:(tt + 1) * 512]
                                .rearrange("(c p) n -> p c n", p=128))

                            pk = psA.tile([128, 512], F32, name="pk", tag="pk")
                            pv = psA.tile([128, 512], F32, name="pv", tag="pv")
                            pq = (psS.tile([128, 512], F32, name="s", tag="s")
                                  if tt == 0 else None)
                            for c in range(CC):
                                nc.tensor.matmul(pk[:], w_s["k", b][:, c, :],
                                                 xtile[:, c, :], start=(c == 0),
                                                 stop=(c == CC - 1))
                                nc.tensor.matmul(pv[:], w_s["v", b][:, c, :],
                                                 xtile[:, c, :], start=(c == 0),
                                                 stop=(c == CC - 1))
                                if tt == 0:
                                    nc.tensor.matmul(
                                        pq[:], w_s["q", b][:, c, :],
                                        xtile[:, c, :], start=(c == 0),
                                        stop=(c == CC - 1))

                            if tt == 0:
                                rq = inv_norm(pq)
                                nc.vector.tensor_mul(qN[b][:], pq[:],
                                                     bcast(rq)[:])

                            tsl = slice(tt * 4, (tt + 1) * 4)

                            # k: fold SCALE (and cls_score) and 1/|k| in
                            rk = inv_norm(pk)
                            fk = htmp.tile([1, 512], F32, name="fk", tag="fk")
                            nc.vector.tensor_scalar_mul(fk[:], rk[:], SCALE)
                            if b == "cls":
                                nc.vector.tensor_mul(
                                    fk[:], fk[:],
                                    score_s[:, tt * 512:(tt + 1) * 512])
                            nc.vector.tensor_mul(kS[b][:, tsl, :], pk[:],
                                                 bcast(fk)[:])

                            # v: normalized + raw + token-major transposes
                            rv = inv_norm(pv)
                            vN_sb = htmp.tile([128, 512], F16, name="vN",
                                              tag="vN")
                            nc.vector.tensor_mul(vN_sb[:], pv[:], bcast(rv)[:])
                            nc.sync.dma_start(
                                vn_d[ib][:, h, tt * 512:(tt + 1) * 512],
                                vN_sb[:])
                            vraw = htmp.tile([128, 512], F32R, name="vraw",
                                             tag="vraw")
                            nc.scalar.activation(vraw[:], pv[:], AF.Copy)
                            if tt == 0:
                                nc.vector.tensor_copy(VG[b][:, h, :], vraw[:])
                            tp4 = psT.tile([128, 4, 128], F32R, name="tp",
                                           tag="tp")
                            for j in range(4):
                                nc.tensor.transpose(
                                    tp4[:, j, :], vraw[:, j * 128:(j + 1) * 128],
                                    ident[:])
                            nc.vector.tensor_copy(vTok[b][:, tsl, :], tp4[:])

                        nc.sync.dma_start(vt_d[ib, h], vTok[b][:])

                    # ---------------- attention ----------------
                    Rhalf = {}
                    for b in B:
                        dacc = psN.tile([1, N1], F32, name="dacc", tag="nsq")
                        for kt in range(KT):
                            s = psS.tile([128, N1], F32, name="s", tag="s")
                            nc.tensor.matmul(s[:], kS[b][:, kt, :], qN[b][:],
                                             start=True, stop=True)
                            p_t = P[b][:, kt, :]
                            nc.scalar.activation(p_t, s[:], AF.Exp)
                            nc.tensor.matmul(dacc[:], ones16[:], p_t,
                                             start=(kt == 0),
                                             stop=(kt == KT - 1))
                        d2 = htmp.tile([1, N1], F32, name="d2", tag="d2")
                        nc.vector.tensor_scalar_mul(d2[:], dacc[:], 2.0)
                        rh = htmp.tile([1, N1], F32, name="rh", tag="rh")
                        nc.vector.reciprocal(rh[:], d2[:])
                        Rhalf[b] = hrow.tile([128, N1], F32, name=f"Rh_{b}",
                                             tag=f"Rh_{b}")
                        nc.gpsimd.partition_broadcast(Rhalf[b][:], rh[:])

                    xacc = {b: psX.tile([128, N1], F32, name=f"x_{b}",
                                        tag=f"x_{b}") for b in B}
                    for kt in range(KT):
                        for b in B:
                            nc.vector.tensor_mul(P[b][:, kt, :], P[b][:, kt, :],
                                                 Rhalf[b][:])
                        # head-sum of the blended attention for the ave
                        # branch: accumulate on the (mostly idle) DMA engines
                        # so the Pool/DVE chains stay off the critical path
                        for b in B:
                            nc.gpsimd.dma_start(avs[:, kt, :], P[b][:, kt, :],
                                                accum_op=mybir.AluOpType.add)
                        for b in B:
                            for i2, b2 in enumerate(B):
                                nc.tensor.matmul(
                                    xacc[b][:], vTok[b][:, kt, :],
                                    P[b2][:, kt, :],
                                    start=(kt == 0 and i2 == 0),
                                    stop=(kt == KT - 1 and i2 == 1))
                    for b in B:
                        nc.vector.tensor_copy(XG[b][:, h, :], xacc[b][:])

            # =========== raw value-similarity masks ===========
            with tc.tile_pool(name="cpool", bufs=1) as cpool:
                msk = {b: cpool.tile([128, KT, N1], F16, name=f"msk_{b}",
                                     tag=f"msk_{b}") for b in B}
                with tc.tile_pool(name="ckey", bufs=3) as ckey, \
                     tc.tile_pool(name="rawps", bufs=2, space="PSUM") as rawps:
                    for ib, (b, thr) in enumerate((("cls", 0.75),
                                                   ("reg", 0.99))):
                        vnq = cpool.tile([128, H, N1], F16, name=f"vnq_{b}",
                                         tag=f"vnq_{b}")
                        nc.sync.dma_start(vnq[:], vn_d[ib][:, :, 0:N1])
                        for kt in range(KT):
                            vnk = ckey.tile([128, H, 128], F16, name="vnk",
                                            tag="vnk")
                            nc.sync.dma_start(
                                vnk[:],
                                vn_d[ib][:, :, kt * 128:(kt + 1) * 128])
                            rp = rawps.tile([128, N1], F32, name="raw",
                                            tag="raw")
                            for hh in range(H):
                                nc.tensor.matmul(rp[:], vnk[:, hh, :],
                                                 vnq[:, hh, :],
                                                 start=(hh == 0),
                                                 stop=(hh == H - 1))
                            nc.vector.tensor_scalar(
                                msk[b][:, kt, :], rp[:], 1.0 / H, thr,
                                mybir.AluOpType.mult, mybir.AluOpType.is_gt)

                # ---- masked exp of the head-averaged attention ----
                mes = cpool.tile([128, KT, N1], BF16, name="mes")
                meo = cpool.tile([128, KT, N1], BF16, name="meo")
                Rd = {}
                with tc.tile_pool(name="dps", bufs=1, space="PSUM") as dps:
                    dp = {b: dps.tile([1, N1], F32, name=f"dp_{b}",
                                      tag=f"dp_{b}") for b in B}
                    for kt in range(KT):
                        e_t = cpool.tile([128, N1], F32R, name="e_t",
                                         tag="e_t")
                        nc.scalar.activation(e_t[:], avs[:, kt, :], AF.Exp,
                                             scale=1.0 / H)
                        nc.vector.tensor_mul(mes[:, kt, :], e_t[:],
                                             msk["cls"][:, kt, :])
                        nc.vector.tensor_mul(meo[:, kt, :], mes[:, kt, :],
                                             msk["reg"][:, kt, :])
                        nc.tensor.matmul(dp["cls"][:], ones16[:],
                                         mes[:, kt, :],
                                         start=(kt == 0), stop=(kt == KT - 1))
                        nc.tensor.matmul(dp["reg"][:], ones16[:],
                                         meo[:, kt, :],
                                         start=(kt == 0), stop=(kt == KT - 1))
                    for b in B:
                        rr = cpool.tile([1, N1], F32, name=f"rr_{b}",
                                        tag=f"rr_{b}")
                        nc.vector.reciprocal(rr[:], dp[b][:])
                        Rd[b] = cpool.tile([128, N1], F32, name=f"Rd_{b}",
                                           tag=f"Rd_{b}")
                        nc.gpsimd.partition_broadcast(Rd[b][:], rr[:])

                # =========== output linears ===========
                with tc.tile_pool(name="lw", bufs=3) as lw, \
                     tc.tile_pool(name="linps", bufs=3, space="PSUM") as linps, \
                     tc.tile_pool(name="ltmp", bufs=3) as ltmp:
                    for ib, b in enumerate(B):
                        for m in range(16):
                            wl = lw.tile([128, CL, 128], F16, name="wl",
                                         tag="wl")
                            base = (ib * 16 + m) * 128
                            nc.sync.dma_start(wl[:],
                                              wlin_ap[base:base + 128, :])
                            op_ = linps.tile([128, N1], F32, name="olin",
                                             tag="olin")
                            for c in range(CL):
                                rhs = (XG[b][:, c, :] if c < CC
                                       else VG[b][:, c - CC, :])
                                nc.tensor.matmul(op_[:], wl[:, c, :], rhs,
                                                 start=(c == 0),
                                                 stop=(c == CL - 1))
                            osb = ltmp.tile([128, N1], F32, name="osb",
                                            tag="osb")
                            nc.vector.tensor_scalar_add(
                                osb[:], op_[:],
                                bias_s[:, ib * 16 + m:ib * 16 + m + 1])
                            r0 = O_LIN[b] + m * 128
                            nc.sync.dma_start(oap[r0:r0 + 128, :], osb[:])

                    # =========== ave branch ===========
                    MS = {"cls": mes, "reg": meo}
                    for ib, b in enumerate(B):
                        for h in range(H):
                            vt = lw.tile([128, KT, 128], BF16, name="vt",
                                         tag="vt")
                            nc.sync.dma_start(vt[:], vt_d[ib, h][:])
                            ap_ = linps.tile([128, N1], F32, name="avep",
                                             tag="avep")
                            for kt in range(KT):
                                nc.tensor.matmul(ap_[:], vt[:, kt, :],
                                                 MS[b][:, kt, :],
                                                 start=(kt == 0),
                                                 stop=(kt == KT - 1))
                            asb = ltmp.tile([128, N1], F32, name="asb",
                                            tag="asb")
                            nc.vector.tensor_mul(asb[:], ap_[:], Rd[b][:])
                            r0 = O_AVE[b] + h * 128
                            nc.sync.dma_start(oap[r0:r0 + 128, :], asb[:])

    nc.finalize()
    return nc


def make_consts(inputs: dict) -> dict:
    """Host-side: pre-lay all weights into const arrays baked into the NEFF."""
    W_q = {"cls": np.asarray(inputs["W_q_cls"], np.float32),
           "reg": np.asarray(inputs["W_q_reg"], np.float32)}
    W_kv = {"cls": np.asarray(inputs["W_kv_cls"], np.float32),
            "reg": np.asarray(inputs["W_kv_reg"], np.float32)}
    W_l = {"cls": np.asarray(inputs["W_lin"], np.float32),
           "reg": np.asarray(inputs["W_lin_reg"], np.float32)}
    b_l = {"cls": np.asarray(inputs["b_lin"], np.float32),
           "reg": np.asarray(inputs["b_lin_reg"], np.float32)}

    wqkv = np.zeros((H * 6 * 128, CC * 128), np.float16)
    for h in range(H):
        hs = slice(h * HD, (h + 1) * HD)
        vs = slice(C + h * HD, C + (h + 1) * HD)
        for (t, b), s in W_SLOT.items():
            src = (W_q[b][:, hs] if t == "q" else
                   W_kv[b][:, hs] if t == "k" else W_kv[b][:, vs])   # [C, 128]
            lay = src.reshape(CC, 128, 128).transpose(1, 0, 2)       # [p, c, m]
            wqkv[(h * 6 + s) * 128:(h * 6 + s + 1) * 128] = \
                lay.reshape(128, CC * 128).astype(np.float16)

    wlin = np.zeros((2 * 16 * 128, CL * 128), np.float16)
    for ib, b in enumerate(B):
        for m in range(16):
            src = W_l[b][:, m * 128:(m + 1) * 128]                   # [2C, 128]
            lay = src.reshape(CL, 128, 128).transpose(1, 0, 2)       # [p, c, u]
            wlin[(ib * 16 + m) * 128:(ib * 16 + m + 1) * 128] = \
                lay.reshape(128, CL * 128).astype(np.float16)

    bias = np.zeros((128, 32), np.float32)
    for ib, b in enumerate(B):
        bias[:, ib * 16:(ib + 1) * 16] = b_l[b].reshape(16, 128).T

    return {"wqkv": wqkv, "wlin": wlin, "bias": bias}


def make_in_maps(inputs: dict) -> list[dict]:
    """Host-side staging: full x^T (f16) + cls_score, single core."""
    x_cls = np.asarray(inputs["x_cls"], np.float32)[0]      # [N2, C]
    x_reg = np.asarray(inputs["x_reg"], np.float32)[0]
    scr = np.asarray(inputs["cls_score"], np.float32).reshape(8, 256)
    xin = np.concatenate([np.ascontiguousarray(x_cls.T),
                          np.ascontiguousarray(x_reg.T)], 0).astype(np.float16)
    return [{"xin": xin, "scr": scr}]


def assemble(results: list[dict]) -> tuple[np.ndarray, np.ndarray]:
    out = results[0]["out"]
    cls_feature = np.ascontiguousarray(out[0:3072].T, dtype=np.float32)
    reg_feature = np.ascontiguousarray(out[3072:6144].T, dtype=np.float32)
    return cls_feature, reg_feature


_CACHE = {}


def _weights_digest(inputs: dict) -> str:
    hsh = hashlib.sha1()
    for k in ("W_q_cls", "W_kv_cls", "W_q_reg", "W_kv_reg",
              "W_lin", "b_lin", "W_lin_reg", "b_lin_reg"):
        hsh.update(np.ascontiguousarray(np.asarray(inputs[k], np.float32)).tobytes())
    return hsh.hexdigest()


def get_nc(inputs: dict | None = None):
    if inputs is not None:
        dig = _weights_digest(inputs)
        if _CACHE.get("digest") != dig:
            _CACHE.clear()
            _CACHE["digest"] = dig
            _CACHE["nc"] = build_nc(make_consts(inputs))
    return _CACHE["nc"]


class _Runner:
    """Cached jitted SPMD executor (mirrors bass2jax.run_bass_via_pjrt)."""

    def __init__(self, nc):
        import jax
        from jax.sharding import Mesh, PartitionSpec
        from jax.experimental.shard_map import shard_map
        from concourse.bass2jax import (_bass_exec_p, install_neuronx_cc_hook,
                                        partition_id_tensor)
        install_neuronx_cc_hook()
        self.jax = jax
        pname = nc.partition_id_tensor.name if nc.partition_id_tensor else None
        in_names, out_names, out_avals, zero_outs = [], [], [], []
        for alloc in nc.m.functions[0].allocations:
            if not isinstance(alloc, mybir.MemoryLocationSet):
                continue
            name = alloc.memorylocations[0].name
            if alloc.kind == "ExternalInput":
                if name != pname:
                    in_names.append(name)
            elif alloc.kind == "ExternalOutput":
                out_names.append(name)
                shape = tuple(alloc.tensor_shape)
                dtype = mybir.dt.np(alloc.dtype)
                out_avals.append(jax.core.ShapedArray(shape, dtype))
                zero_outs.append(np.zeros(shape, dtype))
        self.in_names, self.out_names = in_names, out_names
        self.out_avals, self.zero_outs = out_avals, zero_outs
        n_params, n_outs = len(in_names), len(out_names)
        all_in = in_names + out_names + ([pname] if pname else [])

        def _body(*args):
            operands = list(args)
            if pname is not None:
                operands.append(partition_id_tensor())
            return tuple(_bass_exec_p.bind(
                *operands, out_avals=tuple(out_avals), in_names=tuple(all_in),
                out_names=tuple(out_names), lowering_input_output_aliases=(),
                sim_require_finite=True, sim_require_nnan=True, nc=nc))

        devices = jax.devices()[:N_CORES]
        mesh = Mesh(np.asarray(devices), ("core",))
        self.fn = jax.jit(
            shard_map(_body, mesh=mesh,
                      in_specs=(PartitionSpec("core"),) * (n_params + n_outs),
                      out_specs=(PartitionSpec("core"),) * n_outs,
                      check_rep=False),
            keep_unused=True)

    def __call__(self, in_maps):
        n = N_CORES
        concat_in = [np.concatenate([np.asarray(in_maps[c][k]) for c in range(n)], 0)
                     for k in self.in_names]
        concat_zeros = [np.zeros((n * z.shape[0], *z.shape[1:]), z.dtype)
                        for z in self.zero_outs]
        outs = self.fn(*concat_in, *concat_zeros)
        self.jax.block_until_ready(outs)
        return [{name: np.asarray(outs[i]).reshape(n, *self.out_avals[i].shape)[c]
                 for i, name in enumerate(self.out_names)}
                for c in range(n)]


def get_runner():
    if "runner" not in _CACHE:
        _CACHE["runner"] = _Runner(get_nc())
    return _CACHE["runner"]


def kernel(**inputs) -> tuple[np.ndarray, np.ndarray]:
    get_nc(inputs)
    results = get_runner()(make_in_maps(inputs))
    return assemble(results)
